# revision 8
# baseline (speedup 1.0000x reference)
"""Asymmetric L1 loss on 8 Trainium2 NeuronCores.

reference: loss = sum(where(d<0, -penalty[j]*d, d)) / N  with d = computed - target.

Identity used: where(d<0, -p*d, d) = a_j*d + b_j*|d| with a=(1-p)/2, b=(1+p)/2.
So each core only needs per-column partial sums  B_j = sum(d),  A_j = sum(|d|)
over its shard; the penalty weighting + final scalar reduction happen on host
during the gather/unshard step (512 floats total).

Device kernel per core (data-parallel over rows):
  - SWDGE cast-DMA c,t tiles f32(HBM) -> bf16(SBUF) [128, TILE_F]
    (HBM reads stay f32 = full memory traffic; cast is free in the SDMA
     datapath; bf16 halves downstream engine cost — fp32 matmuls would
     run as HI/LO 2-pass on PE, which made TensorE the 95%-busy
     bottleneck in v1)
  - DVE:  d = c - t            (bf16 2x mode)
  - ACT:  a = |d|
  - PE :  ones[128,1].T @ d-chunk  -> psum_B[1,512]  (accumulating matmuls)
          ones[128,1].T @ a-chunk  -> psum_A[1,512]
    (free index f of psum maps to column j = f % 32 — preserved because all
     chunk offsets are multiples of 32; PSUM accumulates fp32 so only the
     bf16 quantization of d enters the error, ~1e-7 on the final sums)
  - epilogue: strided tensor_reduce folds [1,512] -> [1,32] per accumulator,
    DMA out [1,64] = [B_j | A_j] in f32.
"""

import os
import sys

import numpy as np

if "/opt/trn_rl_repo" not in sys.path:
    sys.path.insert(0, "/opt/trn_rl_repo")

N_TOTAL = 4194304
M_COLS = 32
N_CORES = 8
N_PER_CORE = N_TOTAL // N_CORES          # 524288 rows per core
FLAT = N_PER_CORE * M_COLS               # 16777216 f32 per tensor per core
P = 128
PER_PART = FLAT // P                     # 131072 f32 per partition
TILE_F = 8192                            # free-dim elements per tile (4MiB f32 read per DMA)
MM_F = 512                               # matmul chunk (PSUM bank = 512 f32)

_cache = {}
_last_results = None


def build(n_tiles=PER_PART // TILE_F, per_part=PER_PART, tile_f=TILE_F):
    from concourse import bacc, mybir, tile

    assert n_tiles * tile_f == per_part
    mm_per_tile = tile_f // MM_F

    nc = bacc.Bacc(None, target_bir_lowering=False)
    f32 = mybir.dt.float32
    bf16 = mybir.dt.bfloat16
    c_dram = nc.declare_dram_parameter("computed", [P, per_part], f32, isOutput=False)
    t_dram = nc.declare_dram_parameter("target", [P, per_part], f32, isOutput=False)
    out_dram = nc.declare_dram_parameter("out", [1, 64], f32, isOutput=True)

    with tile.TileContext(nc) as tc:
        with (
            tc.tile_pool(name="cpool", bufs=4) as cpool,
            tc.tile_pool(name="tpool", bufs=4) as tpool,
            tc.tile_pool(name="kpool", bufs=1) as kpool,
            tc.tile_pool(name="fpool", bufs=1) as fpool,
            tc.tile_pool(name="psum", bufs=1, space="PSUM") as psum_pool,
        ):
            ones = kpool.tile([P, 1], bf16)
            nc.vector.memset(ones[:], 1.0)
            psum_b = psum_pool.tile([1, MM_F], f32, tag="pb")
            psum_a = psum_pool.tile([1, MM_F], f32, tag="pa")

            for k in range(n_tiles):
                c = cpool.tile([P, tile_f], bf16, tag="c")
                t = tpool.tile([P, tile_f], bf16, tag="t")
                # gpsimd (SWDGE) path: dtype-casting DMA
                nc.gpsimd.dma_start(out=c[:], in_=c_dram[:, k * tile_f:(k + 1) * tile_f])
                nc.gpsimd.dma_start(out=t[:], in_=t_dram[:, k * tile_f:(k + 1) * tile_f])
                # in-place: d = c - t overwrites c; a = |d| overwrites t
                nc.vector.tensor_sub(out=c[:], in0=c[:], in1=t[:])
                nc.scalar.activation(
                    out=t[:], in_=c[:], func=mybir.ActivationFunctionType.Abs
                )
                for m in range(mm_per_tile):
                    first = k == 0 and m == 0
                    last = k == n_tiles - 1 and m == mm_per_tile - 1
                    nc.tensor.matmul(
                        psum_b[:], ones[:], c[:, m * MM_F:(m + 1) * MM_F],
                        start=first, stop=last,
                    )
                    nc.tensor.matmul(
                        psum_a[:], ones[:], t[:, m * MM_F:(m + 1) * MM_F],
                        start=first, stop=last,
                    )

            res = fpool.tile([1, 64], f32)
            nc.vector.tensor_reduce(
                out=res[:, 0:32],
                in_=psum_b[:].rearrange("p (r j) -> p j r", j=M_COLS),
                axis=mybir.AxisListType.X,
                op=mybir.AluOpType.add,
            )
            nc.vector.tensor_reduce(
                out=res[:, 32:64],
                in_=psum_a[:].rearrange("p (r j) -> p j r", j=M_COLS),
                axis=mybir.AxisListType.X,
                op=mybir.AluOpType.add,
            )
            nc.sync.dma_start(out=out_dram[:], in_=res[:])

    nc.compile()
    return nc


def kernel(computed, target, penalty):
    global _last_results
    from concourse.bass_utils import run_bass_kernel_spmd

    if "nc" not in _cache:
        _cache["nc"] = build()
    nc = _cache["nc"]

    computed = np.ascontiguousarray(computed, dtype=np.float32)
    target = np.ascontiguousarray(target, dtype=np.float32)
    in_maps = []
    for i in range(N_CORES):
        sl = slice(i * N_PER_CORE, (i + 1) * N_PER_CORE)
        in_maps.append(
            {
                "computed": computed[sl].reshape(P, PER_PART),
                "target": target[sl].reshape(P, PER_PART),
            }
        )

    trace = bool(os.environ.get("KERNEL_TRACE"))
    res = run_bass_kernel_spmd(nc, in_maps, core_ids=list(range(N_CORES)), trace=trace)
    _last_results = res

    B = np.zeros(M_COLS, np.float64)
    A = np.zeros(M_COLS, np.float64)
    for r in res.results:
        out = np.asarray(r["out"]).reshape(64).astype(np.float64)
        B += out[:32]
        A += out[32:]
    p = np.asarray(penalty, dtype=np.float64)
    aw = (1.0 - p) / 2.0
    bw = (1.0 + p) / 2.0
    total = float(aw @ B + bw @ A)
    return np.float32(total / N_TOTAL)
